# revision 13
# baseline (speedup 1.0000x reference)
"""MemoryBank (vq_codebook) Trainium2 kernel — v3, transposed-layout.

Computes, for H:(B,128,T) f32 and units:(128,512) f32:
    C[b,s,t] = softmax_s(-||H[b,:,t] - units[:,s]||^2)
Output: (B, 512, T) f32.

Math: softmax_s(-(h2 - 2 h.u + m2)) == softmax_s(2 h.u - m2)  (h2 const in s).

v3 strategy (8 NeuronCores, data-parallel over batch, 4 batches/core):
  Compute directly in the OUTPUT (s, t) layout: for each 128-unit chunk i,
    PSUM(s_local, t) = ones1.T @ (-K/2 row)          (per-token overflow shift)
                     + Uhi_i.T @ Hhi + Uhi_i.T @ Hlo + Ulo_i.T @ Hhi
  (bf16 hi/lo splits; 1024-token moving operand). Then
    e_i = exp(2*PSUM + bias_i)   on ACT, bias_i = -m2[128i+p] per-partition,
                                  written as float32r,
    Zrow = sum_s e  via 8 accumulating fp32r ones-matmuls -> (1, 1024) PSUM,
    Zrow -> SBUF on ACT, partition_broadcast on GpSimd, reciprocal on DVE,
    p_i = e_i * (1/Z)  on DVE  -> big strided DMA to (s, t) DRAM layout.
  No PE transposes, no PSUM->SBUF copies of the big tensor; the per-token
  shift K cancels exactly in softmax so only its range matters.
"""
import numpy as np
import ml_dtypes

B, DIM, T, SIZE = 32, 128, 4096, 512
N_CORES = 8
BPC = B // N_CORES          # batches per core
GT = 1024                   # tokens per group (bf16 moving-operand max)
N_GROUPS = T // GT          # 4 groups per batch
N_CHUNK = SIZE // 128       # 4 unit chunks

_bf = ml_dtypes.bfloat16


def _build_nc():
    import concourse.bacc as bacc
    import concourse.tile as tile
    from concourse import mybir

    f32 = mybir.dt.float32
    f32r = mybir.dt.float32r
    bf16 = mybir.dt.bfloat16

    nc = bacc.Bacc("TRN2", target_bir_lowering=False, debug=False,
                   num_devices=N_CORES)

    Hhi = nc.dram_tensor("Hhi", [BPC, DIM, T], bf16, kind="ExternalInput")
    Hlo = nc.dram_tensor("Hlo", [BPC, DIM, T], bf16, kind="ExternalInput")
    Krow = nc.dram_tensor("Krow", [BPC, 1, T], bf16, kind="ExternalInput")
    Uhi = nc.dram_tensor("Uhi", [DIM, SIZE], bf16, kind="ExternalInput")
    Ulo = nc.dram_tensor("Ulo", [DIM, SIZE], bf16, kind="ExternalInput")
    M2col = nc.dram_tensor("M2col", [DIM, N_CHUNK], f32, kind="ExternalInput")
    Ones1 = nc.dram_tensor("Ones1", [1, 128], bf16, kind="ExternalInput")
    Ones128 = nc.dram_tensor("Ones128", [128, 1], f32r, kind="ExternalInput")
    O = nc.dram_tensor("O", [BPC, SIZE, T], f32, kind="ExternalOutput")

    with tile.TileContext(nc) as tc:
        with (
            tc.tile_pool(name="consts", bufs=1) as consts,
            tc.tile_pool(name="hbuf", bufs=2) as hbuf,
            tc.tile_pool(name="ebuf", bufs=2) as ebuf,
            tc.tile_pool(name="zbuf", bufs=2) as zbuf,
            tc.tile_pool(name="pbuf", bufs=2) as pbuf,
            tc.tile_pool(name="psc", bufs=3, space="PSUM") as psc,
            tc.tile_pool(name="psz", bufs=1, space="PSUM") as psz,
        ):
            uhi = consts.tile([DIM, SIZE], bf16)
            ulo = consts.tile([DIM, SIZE], bf16)
            m2c = consts.tile([DIM, N_CHUNK], f32)
            ones1 = consts.tile([1, 128], bf16)
            ones128 = consts.tile([128, 1], f32r)
            nc.sync.dma_start(uhi[:], Uhi[:])
            nc.sync.dma_start(ulo[:], Ulo[:])
            nc.sync.dma_start(m2c[:], M2col[:])
            nc.sync.dma_start(ones1[:], Ones1[:])
            nc.sync.dma_start(ones128[:], Ones128[:])

            # software-pipelined Z-matmuls: emit group g's Z work after
            # group g+1's chunk matmuls so PE never stalls on ACT.
            pending = None  # (e_all, zp0, zp1, p_all, b, g)

            def emit_znorm(st):
                e_all, zp0, zp1, p_all, b_, g_ = st
                for h, zp in enumerate((zp0, zp1)):
                    for i in range(N_CHUNK):
                        nc.tensor.matmul(
                            zp[:], ones128[:],
                            e_all[:, i, h * 512:(h + 1) * 512],
                            start=(i == 0), stop=(i == N_CHUNK - 1))
                zrow = zbuf.tile([1, GT], f32, tag="zrow")
                nc.scalar.copy(zrow[:, 0:512], zp0[:])
                nc.scalar.copy(zrow[:, 512:1024], zp1[:])
                zb = zbuf.tile([128, GT], f32, tag="zb")
                nc.gpsimd.partition_broadcast(zb[:], zrow[:])
                zr = zbuf.tile([128, GT], f32, tag="zr")
                nc.vector.reciprocal(zr[:], zb[:])
                for i in range(N_CHUNK):
                    nc.vector.tensor_mul(p_all[:, i, :], e_all[:, i, :], zr[:])
                ts0 = g_ * GT
                og = O[b_][:, ts0:ts0 + GT]
                nc.sync.dma_start(
                    og.rearrange("(i p) t -> p i t", i=N_CHUNK), p_all[:])

            for b in range(BPC):
                hhi = hbuf.tile([DIM, T], bf16, tag="hhi")
                hlo = hbuf.tile([DIM, T], bf16, tag="hlo")
                krow = hbuf.tile([1, T], bf16, tag="krow")
                nc.sync.dma_start(hhi[:], Hhi[b][:])
                nc.sync.dma_start(hlo[:], Hlo[b][:])
                nc.sync.dma_start(krow[:], Krow[b][:])
                for g in range(N_GROUPS):
                    ts = slice(g * GT, (g + 1) * GT)
                    e_all = ebuf.tile([128, N_CHUNK, GT], f32r, tag="e")
                    for i in range(N_CHUNK):
                        ci = slice(i * 128, (i + 1) * 128)
                        pc = psc.tile([128, GT], f32)
                        # matmul out must fit one PSUM bank -> two 512 halves
                        for h in range(2):
                            hs = slice(g * GT + h * 512, g * GT + h * 512 + 512)
                            po = pc[:, h * 512:(h + 1) * 512]
                            nc.tensor.matmul(po, ones1[:], krow[:, hs],
                                             start=True, stop=False)
                            nc.tensor.matmul(po, uhi[:, ci], hhi[:, hs],
                                             start=False, stop=False)
                            nc.tensor.matmul(po, uhi[:, ci], hlo[:, hs],
                                             start=False, stop=False)
                            nc.tensor.matmul(po, ulo[:, ci], hhi[:, hs],
                                             start=False, stop=True)
                        nc.scalar.activation(
                            e_all[:, i, :], pc[:],
                            mybir.ActivationFunctionType.Exp,
                            scale=2.0, bias=m2c[:, i:i + 1])
                        if i == 1 and pending is not None:
                            emit_znorm(pending)
                            pending = None
                    zp0 = psz.tile([1, 512], f32, tag="zp0")
                    zp1 = psz.tile([1, 512], f32, tag="zp1")
                    p_all = pbuf.tile([128, N_CHUNK, GT], f32, tag="p")
                    pending = (e_all, zp0, zp1, p_all, b, g)
            emit_znorm(pending)
    nc.compile()
    return nc


_NC_CACHE = []


def _prepare_in_maps(H, units):
    H = np.ascontiguousarray(np.asarray(H, dtype=np.float32))
    units = np.ascontiguousarray(np.asarray(units, dtype=np.float32))

    # host-side input prep (layout/dtype transforms + small stats)
    Hhi = H.astype(_bf)
    Hlo = (H - Hhi.astype(np.float32)).astype(_bf)
    Uhi = units.astype(_bf)
    Ulo = (units - Uhi.astype(np.float32)).astype(_bf)

    m2_64 = (units.astype(np.float64) ** 2).sum(axis=0)      # (SIZE,)
    # per-partition ACT bias: -m2[128*i + p]
    M2col = np.ascontiguousarray(
        (-m2_64.astype(np.float32)).reshape(N_CHUNK, 128).T)  # (128, N_CHUNK)

    # Per-token softmax shift K (cancels exactly; only range matters).
    sh = H.sum(axis=1)                                        # (B, T)
    hn = np.sqrt((H.astype(np.float64) ** 2).sum(axis=1))
    K = (sh + 1.732 * hn - (m2_64.min() + 5.0) + 20.0).astype(np.float32)
    Krow = (-0.5 * K)[:, None, :].astype(_bf)                 # (B, 1, T)

    Ones1 = np.ones((1, 128), dtype=_bf)
    Ones128 = np.ones((128, 1), dtype=np.float32)

    in_maps = []
    for c in range(N_CORES):
        sl = slice(c * BPC, (c + 1) * BPC)
        in_maps.append({
            "Hhi": np.ascontiguousarray(Hhi[sl]),
            "Hlo": np.ascontiguousarray(Hlo[sl]),
            "Krow": np.ascontiguousarray(Krow[sl]),
            "Uhi": Uhi, "Ulo": Ulo, "M2col": M2col,
            "Ones1": Ones1, "Ones128": Ones128,
        })
    return in_maps


def kernel(H, units):
    from concourse.bass_utils import run_bass_kernel_spmd

    in_maps = _prepare_in_maps(H, units)
    if not _NC_CACHE:
        _NC_CACHE.append(_build_nc())
    nc = _NC_CACHE[0]

    res = run_bass_kernel_spmd(nc, in_maps, core_ids=list(range(N_CORES)))
    out = np.concatenate([r["O"] for r in res.results], axis=0)
    return np.ascontiguousarray(out.astype(np.float32))


# revision 19
# speedup vs baseline: 1.1213x; 1.1213x over previous
"""MemoryBank (vq_codebook) Trainium2 kernel — v3, transposed-layout.

Computes, for H:(B,128,T) f32 and units:(128,512) f32:
    C[b,s,t] = softmax_s(-||H[b,:,t] - units[:,s]||^2)
Output: (B, 512, T) f32.

Math: softmax_s(-(h2 - 2 h.u + m2)) == softmax_s(2 h.u - m2)  (h2 const in s).

v3 strategy (8 NeuronCores, data-parallel over batch, 4 batches/core):
  Compute directly in the OUTPUT (s, t) layout: for each 128-unit chunk i,
    PSUM(s_local, t) = ones1.T @ (-K/2 row)          (per-token overflow shift)
                     + Uhi_i.T @ Hhi + Uhi_i.T @ Hlo + Ulo_i.T @ Hhi
  (bf16 hi/lo splits; 1024-token moving operand). Then
    e_i = exp(2*PSUM + bias_i)   on ACT, bias_i = -m2[128i+p] per-partition,
                                  written as float32r,
    Zrow = sum_s e  via 8 accumulating fp32r ones-matmuls -> (1, 1024) PSUM,
    Zrow -> SBUF on ACT, partition_broadcast on GpSimd, reciprocal on DVE,
    p_i = e_i * (1/Z)  on DVE  -> big strided DMA to (s, t) DRAM layout.
  No PE transposes, no PSUM->SBUF copies of the big tensor; the per-token
  shift K cancels exactly in softmax so only its range matters.
"""
import numpy as np
import ml_dtypes

B, DIM, T, SIZE = 32, 128, 4096, 512
N_CORES = 8
BPC = B // N_CORES          # batches per core
GT = 1024                   # tokens per group (bf16 moving-operand max)
N_GROUPS = T // GT          # 4 groups per batch
N_CHUNK = SIZE // 128       # 4 unit chunks

_bf = ml_dtypes.bfloat16


def _build_nc():
    import concourse.bacc as bacc
    import concourse.tile as tile
    from concourse import mybir

    f32 = mybir.dt.float32
    f32r = mybir.dt.float32r
    bf16 = mybir.dt.bfloat16

    nc = bacc.Bacc("TRN2", target_bir_lowering=False, debug=False,
                   num_devices=N_CORES)

    Hhi = nc.dram_tensor("Hhi", [BPC, DIM, T], bf16, kind="ExternalInput")
    Hlo = nc.dram_tensor("Hlo", [BPC, DIM, T], bf16, kind="ExternalInput")
    Krow = nc.dram_tensor("Krow", [BPC, 1, T], bf16, kind="ExternalInput")
    Uhi = nc.dram_tensor("Uhi", [DIM, SIZE], bf16, kind="ExternalInput")
    Ulo = nc.dram_tensor("Ulo", [DIM, SIZE], bf16, kind="ExternalInput")
    M2col = nc.dram_tensor("M2col", [DIM, N_CHUNK], f32, kind="ExternalInput")
    Ones1 = nc.dram_tensor("Ones1", [1, 128], bf16, kind="ExternalInput")
    Ones128 = nc.dram_tensor("Ones128", [128, 1], f32r, kind="ExternalInput")
    O = nc.dram_tensor("O", [BPC, SIZE, T], f32, kind="ExternalOutput")

    with tile.TileContext(nc) as tc:
        with (
            tc.tile_pool(name="consts", bufs=1) as consts,
            tc.tile_pool(name="hbuf", bufs=2) as hbuf,
            tc.tile_pool(name="ebuf", bufs=3) as ebuf,
            tc.tile_pool(name="zbuf", bufs=2) as zbuf,
            tc.tile_pool(name="pbuf", bufs=2) as pbuf,
            tc.tile_pool(name="psc", bufs=3, space="PSUM") as psc,
            tc.tile_pool(name="psz", bufs=1, space="PSUM") as psz,
            tc.tile_pool(name="dscratch", bufs=2, space="DRAM") as dscratch,
        ):
            uhi = consts.tile([DIM, SIZE], bf16)
            ulo = consts.tile([DIM, SIZE], bf16)
            m2c = consts.tile([DIM, N_CHUNK], f32)
            ones1 = consts.tile([1, 128], bf16)
            ones128 = consts.tile([128, 1], f32r)
            nc.sync.dma_start(uhi[:], Uhi[:])
            nc.sync.dma_start(ulo[:], Ulo[:])
            nc.sync.dma_start(m2c[:], M2col[:])
            nc.sync.dma_start(ones1[:], Ones1[:])
            nc.sync.dma_start(ones128[:], Ones128[:])

            # software-pipelined Z-matmuls: emit group g's Z work after
            # group g+1's chunk matmuls so PE never stalls on ACT.
            pending = None  # (e_all, zp0, zp1, p_all, b, g)

            def emit_znorm(st):
                import concourse.bass as bass
                e_all, zp, p_all, b_, g_ = st
                for h in range(2):
                    for i in range(N_CHUNK):
                        nc.tensor.matmul(
                            zp[:, h * 512:(h + 1) * 512], ones128[:],
                            e_all[:, i, h * 512:(h + 1) * 512],
                            start=(i == 0), stop=(i == N_CHUNK - 1))
                # 1/Z = exp(-ln Z): both funcs live in one ACT table set
                lrow = zbuf.tile([1, GT], f32, tag="lrow")
                nc.scalar.activation(lrow[:], zp[:],
                                     mybir.ActivationFunctionType.Ln)
                rrow = zbuf.tile([1, GT], f32, tag="rrow")
                nc.scalar.activation(rrow[:], lrow[:],
                                     mybir.ActivationFunctionType.Exp,
                                     scale=-1.0)
                # broadcast 1/Z row to 128 partitions: bounce through a DRAM
                # scratch row, then stride-0 partition read (DRAM APs allow it)
                rd = dscratch.tile([1, GT], f32, tag="rd")
                nc.sync.dma_start(rd[:], rrow[:])
                zr = zbuf.tile([128, GT], f32, tag="zr")
                rap = rd[:]
                bcast = bass.AP(tensor=rap.tensor, offset=rap.offset,
                                ap=[[0, 128]] + list(rap.ap[1:]))
                nc.gpsimd.dma_start(out=zr[:], in_=bcast)
                for i in range(N_CHUNK):
                    nc.vector.tensor_mul(p_all[:, i, :], e_all[:, i, :], zr[:])
                ts0 = g_ * GT
                og = O[b_][:, ts0:ts0 + GT]
                nc.sync.dma_start(
                    og.rearrange("(i p) t -> p i t", i=N_CHUNK), p_all[:])

            for b in range(BPC):
                hhi = hbuf.tile([DIM, T], bf16, tag="hhi")
                hlo = hbuf.tile([DIM, T], bf16, tag="hlo")
                krow = hbuf.tile([1, T], bf16, tag="krow")
                nc.sync.dma_start(hhi[:], Hhi[b][:])
                nc.sync.dma_start(hlo[:], Hlo[b][:])
                nc.sync.dma_start(krow[:], Krow[b][:])
                for g in range(N_GROUPS):
                    ts = slice(g * GT, (g + 1) * GT)
                    e_all = ebuf.tile([128, N_CHUNK, GT], f32r, tag="e")
                    for i in range(N_CHUNK):
                        ci = slice(i * 128, (i + 1) * 128)
                        pc = psc.tile([128, GT], f32)
                        # matmul out must fit one PSUM bank -> two 512 halves
                        for h in range(2):
                            hs = slice(g * GT + h * 512, g * GT + h * 512 + 512)
                            po = pc[:, h * 512:(h + 1) * 512]
                            nc.tensor.matmul(po, ones1[:], krow[:, hs],
                                             start=True, stop=False)
                            nc.tensor.matmul(po, uhi[:, ci], hhi[:, hs],
                                             start=False, stop=False)
                            nc.tensor.matmul(po, uhi[:, ci], hlo[:, hs],
                                             start=False, stop=False)
                            nc.tensor.matmul(po, ulo[:, ci], hhi[:, hs],
                                             start=False, stop=True)
                        nc.scalar.activation(
                            e_all[:, i, :], pc[:],
                            mybir.ActivationFunctionType.Exp,
                            scale=2.0, bias=m2c[:, i:i + 1])
                        if i == 1 and pending is not None:
                            emit_znorm(pending)
                            pending = None
                    zp = psz.tile([1, GT], f32, tag="zp")
                    p_all = pbuf.tile([128, N_CHUNK, GT], f32, tag="p")
                    pending = (e_all, zp, p_all, b, g)
            emit_znorm(pending)
    nc.compile()
    return nc


_NC_CACHE = []


def _prepare_in_maps(H, units):
    H = np.ascontiguousarray(np.asarray(H, dtype=np.float32))
    units = np.ascontiguousarray(np.asarray(units, dtype=np.float32))

    # host-side input prep (layout/dtype transforms + small stats)
    Hhi = H.astype(_bf)
    Hlo = (H - Hhi.astype(np.float32)).astype(_bf)
    Uhi = units.astype(_bf)
    Ulo = (units - Uhi.astype(np.float32)).astype(_bf)

    m2_64 = (units.astype(np.float64) ** 2).sum(axis=0)      # (SIZE,)
    # per-partition ACT bias: -m2[128*i + p]
    M2col = np.ascontiguousarray(
        (-m2_64.astype(np.float32)).reshape(N_CHUNK, 128).T)  # (128, N_CHUNK)

    # Per-token softmax shift K (cancels exactly; only range matters).
    sh = H.sum(axis=1)                                        # (B, T)
    hn = np.sqrt((H.astype(np.float64) ** 2).sum(axis=1))
    K = (sh + 1.732 * hn - (m2_64.min() + 5.0) + 20.0).astype(np.float32)
    Krow = (-0.5 * K)[:, None, :].astype(_bf)                 # (B, 1, T)

    Ones1 = np.ones((1, 128), dtype=_bf)
    Ones128 = np.ones((128, 1), dtype=np.float32)

    in_maps = []
    for c in range(N_CORES):
        sl = slice(c * BPC, (c + 1) * BPC)
        in_maps.append({
            "Hhi": np.ascontiguousarray(Hhi[sl]),
            "Hlo": np.ascontiguousarray(Hlo[sl]),
            "Krow": np.ascontiguousarray(Krow[sl]),
            "Uhi": Uhi, "Ulo": Ulo, "M2col": M2col,
            "Ones1": Ones1, "Ones128": Ones128,
        })
    return in_maps


def kernel(H, units):
    from concourse.bass_utils import run_bass_kernel_spmd

    in_maps = _prepare_in_maps(H, units)
    if not _NC_CACHE:
        _NC_CACHE.append(_build_nc())
    nc = _NC_CACHE[0]

    res = run_bass_kernel_spmd(nc, in_maps, core_ids=list(range(N_CORES)))
    out = np.concatenate([r["O"] for r in res.results], axis=0)
    return np.ascontiguousarray(out.astype(np.float32))


# revision 21
# speedup vs baseline: 1.2033x; 1.0731x over previous
"""MemoryBank (vq_codebook) Trainium2 kernel — v3, transposed-layout.

Computes, for H:(B,128,T) f32 and units:(128,512) f32:
    C[b,s,t] = softmax_s(-||H[b,:,t] - units[:,s]||^2)
Output: (B, 512, T) f32.

Math: softmax_s(-(h2 - 2 h.u + m2)) == softmax_s(2 h.u - m2)  (h2 const in s).

v3 strategy (8 NeuronCores, data-parallel over batch, 4 batches/core):
  Compute directly in the OUTPUT (s, t) layout: for each 128-unit chunk i,
    PSUM(s_local, t) = ones1.T @ (-K/2 row)          (per-token overflow shift)
                     + Uhi_i.T @ Hhi + Uhi_i.T @ Hlo + Ulo_i.T @ Hhi
  (bf16 hi/lo splits; 1024-token moving operand). Then
    e_i = exp(2*PSUM + bias_i)   on ACT, bias_i = -m2[128i+p] per-partition,
                                  written as float32r,
    Zrow = sum_s e  via 8 accumulating fp32r ones-matmuls -> (1, 1024) PSUM,
    Zrow -> SBUF on ACT, partition_broadcast on GpSimd, reciprocal on DVE,
    p_i = e_i * (1/Z)  on DVE  -> big strided DMA to (s, t) DRAM layout.
  No PE transposes, no PSUM->SBUF copies of the big tensor; the per-token
  shift K cancels exactly in softmax so only its range matters.
"""
import numpy as np
import ml_dtypes

B, DIM, T, SIZE = 32, 128, 4096, 512
N_CORES = 8
BPC = B // N_CORES          # batches per core
GT = 1024                   # tokens per group (bf16 moving-operand max)
N_GROUPS = T // GT          # 4 groups per batch
N_CHUNK = SIZE // 128       # 4 unit chunks

_bf = ml_dtypes.bfloat16


def _build_nc():
    import concourse.bacc as bacc
    import concourse.tile as tile
    from concourse import mybir

    # Keep Exp+Ln in one ACT table set (natural_log_exp_and_others) so the
    # table-load inserter doesn't thrash between per-function sets (the set
    # ids passed to the rust pass keep their original positions, only the
    # membership used for set *selection* is filtered).
    _orig_tables = bacc.get_activation_tables

    def _patched_tables(arch):
        t = _orig_tables(arch)
        both = {mybir.ActivationFunctionType.Exp,
                mybir.ActivationFunctionType.Ln}
        out = {}
        for name, fns in t.items():
            if name == "natural_log_exp_and_others":
                out[name] = fns
            else:
                out[name] = fns - both
        return out

    f32 = mybir.dt.float32
    f32r = mybir.dt.float32r
    bf16 = mybir.dt.bfloat16

    nc = bacc.Bacc("TRN2", target_bir_lowering=False, debug=False,
                   num_devices=N_CORES)

    Hhi = nc.dram_tensor("Hhi", [BPC, DIM, T], bf16, kind="ExternalInput")
    Hlo = nc.dram_tensor("Hlo", [BPC, DIM, T], bf16, kind="ExternalInput")
    Krow = nc.dram_tensor("Krow", [BPC, 1, T], bf16, kind="ExternalInput")
    Uhi = nc.dram_tensor("Uhi", [DIM, SIZE], bf16, kind="ExternalInput")
    Ulo = nc.dram_tensor("Ulo", [DIM, SIZE], bf16, kind="ExternalInput")
    M2col = nc.dram_tensor("M2col", [DIM, N_CHUNK], f32, kind="ExternalInput")
    Ones1 = nc.dram_tensor("Ones1", [1, 128], bf16, kind="ExternalInput")
    Ones128 = nc.dram_tensor("Ones128", [128, 1], f32r, kind="ExternalInput")
    O = nc.dram_tensor("O", [BPC, SIZE, T], f32, kind="ExternalOutput")

    with tile.TileContext(nc) as tc:
        with (
            tc.tile_pool(name="consts", bufs=1) as consts,
            tc.tile_pool(name="hbuf", bufs=2) as hbuf,
            tc.tile_pool(name="ebuf", bufs=3) as ebuf,
            tc.tile_pool(name="zbuf", bufs=2) as zbuf,
            tc.tile_pool(name="pbuf", bufs=2) as pbuf,
            tc.tile_pool(name="psc", bufs=3, space="PSUM") as psc,
            tc.tile_pool(name="psz", bufs=1, space="PSUM") as psz,
            tc.tile_pool(name="dscratch", bufs=2, space="DRAM") as dscratch,
        ):
            uhi = consts.tile([DIM, SIZE], bf16)
            ulo = consts.tile([DIM, SIZE], bf16)
            m2c = consts.tile([DIM, N_CHUNK], f32)
            ones1 = consts.tile([1, 128], bf16)
            ones128 = consts.tile([128, 1], f32r)
            nc.sync.dma_start(uhi[:], Uhi[:])
            nc.sync.dma_start(ulo[:], Ulo[:])
            nc.sync.dma_start(m2c[:], M2col[:])
            nc.sync.dma_start(ones1[:], Ones1[:])
            nc.sync.dma_start(ones128[:], Ones128[:])

            # software-pipelined Z-matmuls: emit group g's Z work after
            # group g+1's chunk matmuls so PE never stalls on ACT.
            pending = None  # (e_all, zp0, zp1, p_all, b, g)

            def emit_znorm(st):
                import concourse.bass as bass
                e_all, zp, p_all, b_, g_ = st
                for h in range(2):
                    for i in range(N_CHUNK):
                        nc.tensor.matmul(
                            zp[:, h * 512:(h + 1) * 512], ones128[:],
                            e_all[:, i, h * 512:(h + 1) * 512],
                            start=(i == 0), stop=(i == N_CHUNK - 1))
                # 1/Z = exp(-ln Z): both funcs live in one ACT table set
                lrow = zbuf.tile([1, GT], f32, tag="lrow")
                nc.scalar.activation(lrow[:], zp[:],
                                     mybir.ActivationFunctionType.Ln)
                rrow = zbuf.tile([1, GT], f32, tag="rrow")
                nc.scalar.activation(rrow[:], lrow[:],
                                     mybir.ActivationFunctionType.Exp,
                                     scale=-1.0)
                # broadcast 1/Z row to 128 partitions: bounce through a DRAM
                # scratch row, then stride-0 partition read (DRAM APs allow it)
                rd = dscratch.tile([1, GT], f32, tag="rd")
                nc.sync.dma_start(rd[:], rrow[:])
                zr = zbuf.tile([128, GT], f32, tag="zr")
                rap = rd[:]
                bcast = bass.AP(tensor=rap.tensor, offset=rap.offset,
                                ap=[[0, 128]] + list(rap.ap[1:]))
                nc.gpsimd.dma_start(out=zr[:], in_=bcast)
                for i in range(N_CHUNK):
                    nc.vector.tensor_mul(p_all[:, i, :], e_all[:, i, :], zr[:])
                ts0 = g_ * GT
                og = O[b_][:, ts0:ts0 + GT]
                nc.sync.dma_start(
                    og.rearrange("(i p) t -> p i t", i=N_CHUNK), p_all[:])

            for b in range(BPC):
                hhi = hbuf.tile([DIM, T], bf16, tag="hhi")
                hlo = hbuf.tile([DIM, T], bf16, tag="hlo")
                krow = hbuf.tile([1, T], bf16, tag="krow")
                nc.sync.dma_start(hhi[:], Hhi[b][:])
                nc.sync.dma_start(hlo[:], Hlo[b][:])
                nc.sync.dma_start(krow[:], Krow[b][:])
                for g in range(N_GROUPS):
                    ts = slice(g * GT, (g + 1) * GT)
                    e_all = ebuf.tile([128, N_CHUNK, GT], f32r, tag="e")
                    for i in range(N_CHUNK):
                        ci = slice(i * 128, (i + 1) * 128)
                        pc = psc.tile([128, GT], f32)
                        # matmul out must fit one PSUM bank -> two 512 halves
                        for h in range(2):
                            hs = slice(g * GT + h * 512, g * GT + h * 512 + 512)
                            po = pc[:, h * 512:(h + 1) * 512]
                            nc.tensor.matmul(po, ones1[:], krow[:, hs],
                                             start=True, stop=False)
                            nc.tensor.matmul(po, uhi[:, ci], hhi[:, hs],
                                             start=False, stop=False)
                            nc.tensor.matmul(po, uhi[:, ci], hlo[:, hs],
                                             start=False, stop=False)
                            nc.tensor.matmul(po, ulo[:, ci], hhi[:, hs],
                                             start=False, stop=True)
                        nc.scalar.activation(
                            e_all[:, i, :], pc[:],
                            mybir.ActivationFunctionType.Exp,
                            scale=2.0, bias=m2c[:, i:i + 1])
                        if i == 1 and pending is not None:
                            emit_znorm(pending)
                            pending = None
                    zp = psz.tile([1, GT], f32, tag="zp")
                    p_all = pbuf.tile([128, N_CHUNK, GT], f32, tag="p")
                    pending = (e_all, zp, p_all, b, g)
            emit_znorm(pending)
    bacc.get_activation_tables = _patched_tables
    try:
        nc.compile()
    finally:
        bacc.get_activation_tables = _orig_tables
    return nc


_NC_CACHE = []


def _prepare_in_maps(H, units):
    H = np.ascontiguousarray(np.asarray(H, dtype=np.float32))
    units = np.ascontiguousarray(np.asarray(units, dtype=np.float32))

    # host-side input prep (layout/dtype transforms + small stats)
    Hhi = H.astype(_bf)
    Hlo = (H - Hhi.astype(np.float32)).astype(_bf)
    Uhi = units.astype(_bf)
    Ulo = (units - Uhi.astype(np.float32)).astype(_bf)

    m2_64 = (units.astype(np.float64) ** 2).sum(axis=0)      # (SIZE,)
    # per-partition ACT bias: -m2[128*i + p]
    M2col = np.ascontiguousarray(
        (-m2_64.astype(np.float32)).reshape(N_CHUNK, 128).T)  # (128, N_CHUNK)

    # Per-token softmax shift K (cancels exactly; only range matters).
    sh = H.sum(axis=1)                                        # (B, T)
    hn = np.sqrt((H.astype(np.float64) ** 2).sum(axis=1))
    K = (sh + 1.732 * hn - (m2_64.min() + 5.0) + 20.0).astype(np.float32)
    Krow = (-0.5 * K)[:, None, :].astype(_bf)                 # (B, 1, T)

    Ones1 = np.ones((1, 128), dtype=_bf)
    Ones128 = np.ones((128, 1), dtype=np.float32)

    in_maps = []
    for c in range(N_CORES):
        sl = slice(c * BPC, (c + 1) * BPC)
        in_maps.append({
            "Hhi": np.ascontiguousarray(Hhi[sl]),
            "Hlo": np.ascontiguousarray(Hlo[sl]),
            "Krow": np.ascontiguousarray(Krow[sl]),
            "Uhi": Uhi, "Ulo": Ulo, "M2col": M2col,
            "Ones1": Ones1, "Ones128": Ones128,
        })
    return in_maps


def kernel(H, units):
    from concourse.bass_utils import run_bass_kernel_spmd

    in_maps = _prepare_in_maps(H, units)
    if not _NC_CACHE:
        _NC_CACHE.append(_build_nc())
    nc = _NC_CACHE[0]

    res = run_bass_kernel_spmd(nc, in_maps, core_ids=list(range(N_CORES)))
    out = np.concatenate([r["O"] for r in res.results], axis=0)
    return np.ascontiguousarray(out.astype(np.float32))


# revision 24
# speedup vs baseline: 1.4322x; 1.1902x over previous
"""MemoryBank (vq_codebook) Trainium2 kernel — v3, transposed-layout.

Computes, for H:(B,128,T) f32 and units:(128,512) f32:
    C[b,s,t] = softmax_s(-||H[b,:,t] - units[:,s]||^2)
Output: (B, 512, T) f32.

Math: softmax_s(-(h2 - 2 h.u + m2)) == softmax_s(2 h.u - m2)  (h2 const in s).

v3 strategy (8 NeuronCores, data-parallel over batch, 4 batches/core):
  Compute directly in the OUTPUT (s, t) layout: for each 128-unit chunk i,
    PSUM(s_local, t) = ones1.T @ (-K/2 row)          (per-token overflow shift)
                     + Uhi_i.T @ Hhi + Uhi_i.T @ Hlo + Ulo_i.T @ Hhi
  (bf16 hi/lo splits; 1024-token moving operand). Then
    e_i = exp(2*PSUM + bias_i)   on ACT, bias_i = -m2[128i+p] per-partition,
                                  written as float32r,
    Zrow = sum_s e  via 8 accumulating fp32r ones-matmuls -> (1, 1024) PSUM,
    Zrow -> SBUF on ACT, partition_broadcast on GpSimd, reciprocal on DVE,
    p_i = e_i * (1/Z)  on DVE  -> big strided DMA to (s, t) DRAM layout.
  No PE transposes, no PSUM->SBUF copies of the big tensor; the per-token
  shift K cancels exactly in softmax so only its range matters.
"""
import numpy as np
import ml_dtypes

B, DIM, T, SIZE = 32, 128, 4096, 512
N_CORES = 8
BPC = B // N_CORES          # batches per core
GT = 1024                   # tokens per group (bf16 moving-operand max)
N_GROUPS = T // GT          # 4 groups per batch
N_CHUNK = SIZE // 128       # 4 unit chunks

_bf = ml_dtypes.bfloat16


def _build_nc():
    import concourse.bacc as bacc
    import concourse.tile as tile
    from concourse import mybir

    # Keep Exp+Ln in one ACT table set (natural_log_exp_and_others) so the
    # table-load inserter doesn't thrash between per-function sets (the set
    # ids passed to the rust pass keep their original positions, only the
    # membership used for set *selection* is filtered).
    _orig_tables = bacc.get_activation_tables

    def _patched_tables(arch):
        t = _orig_tables(arch)
        both = {mybir.ActivationFunctionType.Exp,
                mybir.ActivationFunctionType.Ln}
        out = {}
        for name, fns in t.items():
            if name == "natural_log_exp_and_others":
                out[name] = fns
            else:
                out[name] = fns - both
        return out

    f32 = mybir.dt.float32
    f32r = mybir.dt.float32r
    bf16 = mybir.dt.bfloat16

    nc = bacc.Bacc("TRN2", target_bir_lowering=False, debug=False,
                   num_devices=N_CORES)

    Hhi = nc.dram_tensor("Hhi", [BPC, DIM, T], bf16, kind="ExternalInput")
    Hlo = nc.dram_tensor("Hlo", [BPC, DIM, T], bf16, kind="ExternalInput")
    Krow = nc.dram_tensor("Krow", [BPC, 1, T], bf16, kind="ExternalInput")
    Uhi = nc.dram_tensor("Uhi", [DIM, SIZE], bf16, kind="ExternalInput")
    Ulo = nc.dram_tensor("Ulo", [DIM, SIZE], bf16, kind="ExternalInput")
    M2col = nc.dram_tensor("M2col", [DIM, N_CHUNK], f32, kind="ExternalInput")
    Ones1 = nc.dram_tensor("Ones1", [1, 128], bf16, kind="ExternalInput")
    Ones128 = nc.dram_tensor("Ones128", [128, 1], f32r, kind="ExternalInput")
    O = nc.dram_tensor("O", [BPC, SIZE, T], f32, kind="ExternalOutput")

    with tile.TileContext(nc) as tc:
        with (
            tc.tile_pool(name="consts", bufs=1) as consts,
            tc.tile_pool(name="hbuf", bufs=3) as hbuf,
            tc.tile_pool(name="ebuf", bufs=3) as ebuf,
            tc.tile_pool(name="zbuf", bufs=3) as zbuf,
            tc.tile_pool(name="pbuf", bufs=3) as pbuf,
            tc.tile_pool(name="psc", bufs=3, space="PSUM") as psc,
            tc.tile_pool(name="psz", bufs=1, space="PSUM") as psz,
            tc.tile_pool(name="dscratch", bufs=2, space="DRAM") as dscratch,
        ):
            uhi = consts.tile([DIM, SIZE], bf16)
            ulo = consts.tile([DIM, SIZE], bf16)
            m2c = consts.tile([DIM, N_CHUNK], f32)
            ones1 = consts.tile([1, 128], bf16)
            ones128 = consts.tile([128, 1], f32r)
            nc.sync.dma_start(uhi[:], Uhi[:])
            nc.sync.dma_start(ulo[:], Ulo[:])
            nc.sync.dma_start(m2c[:], M2col[:])
            nc.sync.dma_start(ones1[:], Ones1[:])
            nc.sync.dma_start(ones128[:], Ones128[:])

            # software-pipelined Z-matmuls: emit group g's Z work after
            # group g+1's chunk matmuls so PE never stalls on ACT.
            pending = None  # (e_all, zp0, zp1, p_all, b, g)

            def emit_znorm(st):
                import concourse.bass as bass
                e_all, zp, p_all, b_, g_ = st
                for h in range(2):
                    for i in range(N_CHUNK):
                        nc.tensor.matmul(
                            zp[:, h * 512:(h + 1) * 512], ones128[:],
                            e_all[:, i, h * 512:(h + 1) * 512],
                            start=(i == 0), stop=(i == N_CHUNK - 1))
                # 1/Z = exp(-ln Z): both funcs live in one ACT table set
                lrow = zbuf.tile([1, GT], f32, tag="lrow")
                nc.scalar.activation(lrow[:], zp[:],
                                     mybir.ActivationFunctionType.Ln)
                rrow = zbuf.tile([1, GT], f32, tag="rrow")
                nc.scalar.activation(rrow[:], lrow[:],
                                     mybir.ActivationFunctionType.Exp,
                                     scale=-1.0)
                # broadcast 1/Z row to 128 partitions (SBUF-only, on GpSimd)
                zr = zbuf.tile([128, GT], f32, tag="zr")
                nc.gpsimd.partition_broadcast(zr[:], rrow[:])
                for i in range(N_CHUNK):
                    nc.vector.tensor_mul(p_all[:, i, :], e_all[:, i, :], zr[:])
                ts0 = g_ * GT
                og = O[b_][:, ts0:ts0 + GT]
                nc.sync.dma_start(
                    og.rearrange("(i p) t -> p i t", i=N_CHUNK), p_all[:])

            for b in range(BPC):
                krow = hbuf.tile([1, T], bf16, tag="krow")
                nc.sync.dma_start(krow[:], Krow[b][:])
                for g in range(N_GROUPS):
                    ts = slice(g * GT, (g + 1) * GT)
                    hhi = hbuf.tile([DIM, GT], bf16, tag="hhi")
                    hlo = hbuf.tile([DIM, GT], bf16, tag="hlo")
                    nc.sync.dma_start(hhi[:], Hhi[b][:, ts])
                    nc.sync.dma_start(hlo[:], Hlo[b][:, ts])
                    e_all = ebuf.tile([128, N_CHUNK, GT], f32r, tag="e")
                    for i in range(N_CHUNK):
                        ci = slice(i * 128, (i + 1) * 128)
                        pc = psc.tile([128, GT], f32)
                        # matmul out must fit one PSUM bank -> two 512 halves
                        for h in range(2):
                            hs = slice(g * GT + h * 512, g * GT + h * 512 + 512)
                            hl = slice(h * 512, h * 512 + 512)
                            po = pc[:, h * 512:(h + 1) * 512]
                            nc.tensor.matmul(po, ones1[:], krow[:, hs],
                                             start=True, stop=False)
                            nc.tensor.matmul(po, uhi[:, ci], hhi[:, hl],
                                             start=False, stop=False)
                            nc.tensor.matmul(po, uhi[:, ci], hlo[:, hl],
                                             start=False, stop=False)
                            nc.tensor.matmul(po, ulo[:, ci], hhi[:, hl],
                                             start=False, stop=True)
                        nc.scalar.activation(
                            e_all[:, i, :], pc[:],
                            mybir.ActivationFunctionType.Exp,
                            scale=2.0, bias=m2c[:, i:i + 1])
                        if i == 1 and pending is not None:
                            emit_znorm(pending)
                            pending = None
                    zp = psz.tile([1, GT], f32, tag="zp")
                    p_all = pbuf.tile([128, N_CHUNK, GT], f32, tag="p")
                    pending = (e_all, zp, p_all, b, g)
            emit_znorm(pending)
    bacc.get_activation_tables = _patched_tables
    try:
        nc.compile()
    finally:
        bacc.get_activation_tables = _orig_tables
    return nc


_NC_CACHE = []


def _prepare_in_maps(H, units):
    H = np.ascontiguousarray(np.asarray(H, dtype=np.float32))
    units = np.ascontiguousarray(np.asarray(units, dtype=np.float32))

    # host-side input prep (layout/dtype transforms + small stats)
    Hhi = H.astype(_bf)
    Hlo = (H - Hhi.astype(np.float32)).astype(_bf)
    Uhi = units.astype(_bf)
    Ulo = (units - Uhi.astype(np.float32)).astype(_bf)

    m2_64 = (units.astype(np.float64) ** 2).sum(axis=0)      # (SIZE,)
    # per-partition ACT bias: -m2[128*i + p]
    M2col = np.ascontiguousarray(
        (-m2_64.astype(np.float32)).reshape(N_CHUNK, 128).T)  # (128, N_CHUNK)

    # Per-token softmax shift K (cancels exactly; only range matters).
    sh = H.sum(axis=1)                                        # (B, T)
    hn = np.sqrt((H.astype(np.float64) ** 2).sum(axis=1))
    K = (sh + 1.732 * hn - (m2_64.min() + 5.0) + 20.0).astype(np.float32)
    Krow = (-0.5 * K)[:, None, :].astype(_bf)                 # (B, 1, T)

    Ones1 = np.ones((1, 128), dtype=_bf)
    Ones128 = np.ones((128, 1), dtype=np.float32)

    in_maps = []
    for c in range(N_CORES):
        sl = slice(c * BPC, (c + 1) * BPC)
        in_maps.append({
            "Hhi": np.ascontiguousarray(Hhi[sl]),
            "Hlo": np.ascontiguousarray(Hlo[sl]),
            "Krow": np.ascontiguousarray(Krow[sl]),
            "Uhi": Uhi, "Ulo": Ulo, "M2col": M2col,
            "Ones1": Ones1, "Ones128": Ones128,
        })
    return in_maps


def kernel(H, units):
    from concourse.bass_utils import run_bass_kernel_spmd

    in_maps = _prepare_in_maps(H, units)
    if not _NC_CACHE:
        _NC_CACHE.append(_build_nc())
    nc = _NC_CACHE[0]

    res = run_bass_kernel_spmd(nc, in_maps, core_ids=list(range(N_CORES)))
    out = np.concatenate([r["O"] for r in res.results], axis=0)
    return np.ascontiguousarray(out.astype(np.float32))
